# revision 58
# baseline (speedup 1.0000x reference)
"""MAB (multihead attention block) TRN2 kernel, v3.

Sharding: 8 cores = batch (4) x query-half (2). Each core computes its
[1024, 256] output slice with zero cross-core communication (K/V
projections are recomputed by the 2 cores sharing a batch).

Design (driven by the TimelineSim cost model):
- All DRAM tensors are pre-transposed on the HOST so every DMA moves
  contiguous >=1KB runs.
- The Act engine is kept a pure, never-stalling exp stream (the kernel
  is Act-bound): scores run bf16 [k, q]; exp writes fp8 tiles that
  persist in SBUF; A@V replays them as fp8 matmuls with NATURAL
  [q, dh+1] output, a ones-column in V producing softmax denominators.
- Everything else (A@V groups, normalization, residuals, LayerNorms,
  FFN) is emitted as thunks drained one-per-exp-iteration into the
  engine slack under the exp stream. A@V groups for the first head
  pair drain inside their own block's exp stream.
- LayerNorm stats are per-partition scalars in natural layout; rsqrt
  is a Quake-style bit trick + 2 Newton steps on DVE.
- Work is spread across DVE / Pool / Act so no engine's queue gates
  the exp stream.
"""

import numpy as np

import concourse.mybir as mybir
import concourse.tile as tile
from concourse import bacc
from concourse.bass_utils import run_bass_kernel_spmd

F32 = mybir.dt.float32
F32R = mybir.dt.float32r
BF16 = mybir.dt.bfloat16
FP8 = mybir.dt.float8e4
I32 = mybir.dt.int32
AF = mybir.ActivationFunctionType
ALU = mybir.AluOpType

B, NQ, NK, D = 4, 2048, 2048, 256
H, DH = 4, 64
S = NQ // 2          # queries per core
ET = D // 128        # feature tiles
KT = NK // 128       # key chunks of 128
KB = NK // 512       # key blocks of 512
QB = S // 512        # query blocks of 512
TT = S // 128        # token tiles of 128
EPS = 1e-5
SCALE = 1.0 / np.sqrt(D)

_CACHE = {}


def _build(flags):
    (use_bq, use_bk, use_bv, use_bo, use_g0, use_g1) = flags
    nc = bacc.Bacc(None, target_bir_lowering=False)

    dQ = nc.dram_tensor("QT", [D, S], F32, kind="ExternalInput")
    dK = nc.dram_tensor("KT", [D, NK], F32, kind="ExternalInput")
    dW = {w: nc.dram_tensor(w, [D, D], F32, kind="ExternalInput")
          for w in ("WqT", "WkT", "WvT", "WoT")}
    dV = {v: nc.dram_tensor(v, [D], F32, kind="ExternalInput")
          for v in ("bq", "bk", "bv", "bo", "g0", "b0", "g1", "b1")}
    dI = nc.dram_tensor("IDN", [128, 128], F32, kind="ExternalInput")
    dO = nc.dram_tensor("Out", [S, D], F32, kind="ExternalOutput")

    any_flag = any(flags)

    with tile.TileContext(nc) as tc:
        with (
            tc.tile_pool(name="const", bufs=1) as cpool,
            tc.tile_pool(name="acts", bufs=1) as apool,
            tc.tile_pool(name="small", bufs=6) as smp,
            tc.tile_pool(name="stage", bufs=1) as stpool,
        ):
            # ---------------- DMAs (issue order = need order) ----------------
            w0 = {}
            for w in ("WqT", "WkT"):
                w0[w] = cpool.tile([128, ET, D], F32, name=f"{w}0")
                nc.sync.dma_start(w0[w][:], dW[w].rearrange("(dt d) e -> d dt e", d=128))
            QT0 = stpool.tile([128, ET, S], F32)
            KT0 = stpool.tile([128, ET, NK], F32)
            qv = dQ.rearrange("(dt d) s -> d dt s", d=128)
            kv = dK.rearrange("(dt d) k -> d dt k", d=128)
            nc.sync.dma_start(QT0[:, :, 0:512], qv[:, :, 0:512])
            nc.sync.dma_start(KT0[:, :, 0:512], kv[:, :, 0:512])
            nc.sync.dma_start(QT0[:, :, 512:1024], qv[:, :, 512:1024])
            for kb in range(1, KB):
                ksl = slice(kb * 512, (kb + 1) * 512)
                nc.sync.dma_start(KT0[:, :, ksl], kv[:, :, ksl])
            w0["WvT"] = cpool.tile([128, ET, D], F32, name="WvT0")
            nc.sync.dma_start(w0["WvT"][:], dW["WvT"].rearrange("(dt d) e -> d dt e", d=128))
            w0["WoT"] = cpool.tile([128, ET, D], F32, name="WoT0")
            nc.sync.dma_start(w0["WoT"][:], dW["WoT"].rearrange("(dt d) e -> d dt e", d=128))
            idn = cpool.tile([128, 128], F32)
            nc.sync.dma_start(idn[:], dI[:, :])

            vrows = {}
            vcols = {}
            if any_flag:
                onesr1 = cpool.tile([1, 128], F32R)
                o1f = cpool.tile([1, 128], F32)
                nc.vector.memset(o1f[:], 1.0)
                nc.vector.tensor_copy(onesr1[:], o1f[:])
            if use_bq or use_bv:
                # softmax rows sum to 1 => attention(v + bv) == attention(v)
                # + bv; both bq and bv fold into the natural-layout q via a
                # rank-1 matmul term.
                bq_row = cpool.tile([1, D], F32)
                bv_row = cpool.tile([1, D], F32)
                nc.sync.dma_start(bq_row[:], dV["bq"][None, :])
                nc.sync.dma_start(bv_row[:], dV["bv"][None, :])
                bqv = cpool.tile([1, D], F32R)
                s0 = cpool.tile([1, D], F32)
                nc.vector.tensor_tensor(out=s0[:], in0=bq_row[:], in1=bv_row[:], op=ALU.add)
                nc.vector.tensor_copy(bqv[:], s0[:])
                vrows["bqv"] = bqv
            for nm, use in (("bq", use_bq), ("bk", use_bk), ("bo", use_bo)):
                if use:
                    t = cpool.tile([128, ET], F32, name=f"{nm}c")
                    nc.sync.dma_start(t[:], dV[nm].rearrange("(et e) -> e et", e=128))
                    vcols[nm] = t
            for nm, use in (("g0", use_g0), ("b0", use_g0),
                            ("g1", use_g1), ("b1", use_g1)):
                if use:
                    t0 = cpool.tile([1, D], F32, name=f"{nm}row0")
                    nc.sync.dma_start(t0[:], dV[nm][None, :])
                    t = cpool.tile([1, D], F32R, name=f"{nm}row")
                    nc.vector.tensor_copy(t[:], t0[:])
                    vrows[nm] = t

            # ---------------- rounding copies (f32 -> f32r) ----------------
            # DVE: Wq/Wk/Wv (head critical); Pool: Wo (needed late) + K
            # chunks kb1-3; Act: Q + first K chunk (Act idle pre-exp).
            w_r = {}
            for w in ("WqT", "WkT"):
                w_r[w] = cpool.tile([128, ET, D], F32R, name=w)
                nc.vector.tensor_copy(w_r[w][:], w0[w][:])
            QTt = stpool.tile([128, ET, S], F32R)
            KTt = stpool.tile([128, ET, NK], F32R)
            nc.scalar.copy(QTt[:, :, 0:512], QT0[:, :, 0:512])
            nc.scalar.copy(KTt[:, :, 0:512], KT0[:, :, 0:512])
            nc.gpsimd.tensor_copy(KTt[:, :, 512:768], KT0[:, :, 512:768])
            nc.vector.tensor_copy(KTt[:, :, 768:1024], KT0[:, :, 768:1024])
            nc.gpsimd.tensor_copy(QTt[:, :, 512:1024], QT0[:, :, 512:1024])
            for kb in (2, 3):
                ksl = slice(kb * 512, (kb + 1) * 512)
                nc.vector.tensor_copy(KTt[:, :, ksl], KT0[:, :, ksl])
            w_r["WvT"] = cpool.tile([128, ET, D], F32R, name="WvT")
            nc.gpsimd.tensor_copy(w_r["WvT"][:], w0["WvT"][:])
            w_r["WoT"] = cpool.tile([128, ET, D], F32R, name="WoT")
            nc.gpsimd.tensor_copy(w_r["WoT"][:], w0["WoT"][:])

            # ---------------- activation tiles ----------------
            kT_bf = apool.tile([128, ET, NK], BF16)
            qT_bf = apool.tile([128, ET, S], BF16)
            v_sb = apool.tile([128, KT, H, DH + 1], FP8)
            q_nat = apool.tile([128, TT, D], F32)
            O0n = apool.tile([128, TT, D], F32)
            O1n = apool.tile([128, TT, D], F32)
            O2n = apool.tile([128, TT, D], F32)
            O3n = apool.tile([128, TT, D], F32)
            O1T = apool.tile([128, ET, S], F32R)
            rT = apool.tile([128, ET, S], F32)
            scratch = apool.tile([128, D], F32)
            uts = apool.tile([128, 32, 1024], FP8)   # exp tiles, one block

            nc.gpsimd.memset(v_sb[:, :, :, DH:DH + 1], 1.0)

            # ---------------- LN helper (natural layout, per-sub) ----------
            def ln_stats(x, qb, sub, sum4, sumsq4, tail=False):
                ti = qb * 4 + sub
                if tail and sub % 2 == 0:
                    # Act is idle once the exp stream ends: Square/Copy with
                    # the row accumulator produce both stats off-DVE.
                    nc.scalar.activation(scratch[:], x[:, ti, :], AF.Square,
                                         accum_out=sumsq4[:, sub:sub + 1])
                    nc.scalar.activation(scratch[:], x[:, ti, :], AF.Copy,
                                         accum_out=sum4[:, sub:sub + 1])
                else:
                    nc.vector.tensor_tensor(out=scratch[:], in0=x[:, ti, :],
                                            in1=x[:, ti, :], op=ALU.mult)
                    nc.vector.tensor_reduce(out=sumsq4[:, sub:sub + 1], in_=scratch[:],
                                            axis=mybir.AxisListType.X, op=ALU.add)
                    nc.vector.tensor_reduce(out=sum4[:, sub:sub + 1], in_=x[:, ti, :],
                                            axis=mybir.AxisListType.X, op=ALU.add)

            def ln_finish(x, y, qb, gname, bname, use_g, tag, sum4, sumsq4, cps,
                          tail=False, store=False):
                negmu = smp.tile([128, 4], F32, name=f"nmu{tag}", tag="nmu4")
                nc.vector.tensor_scalar_mul(negmu[:], sum4[:], -1.0 / D)
                var4 = smp.tile([128, 4], F32, name=f"var{tag}", tag="var4")
                nc.vector.tensor_tensor(out=var4[:], in0=negmu[:], in1=negmu[:], op=ALU.mult)
                nc.vector.tensor_scalar_mul(sumsq4[:], sumsq4[:], 1.0 / D)
                nc.vector.tensor_tensor(out=var4[:], in0=sumsq4[:], in1=var4[:], op=ALU.subtract)
                nc.vector.tensor_scalar_add(var4[:], var4[:], EPS)
                # quake rsqrt + 2 Newton iterations (keeps Act exp-only)
                yq = smp.tile([128, 4], I32, name=f"yq{tag}", tag="yq4")
                nc.vector.tensor_scalar(yq[:], var4[:].bitcast(I32), 1, None,
                                        ALU.logical_shift_right)
                nc.vector.tensor_scalar(yq[:], yq[:], -1, None, ALU.bitwise_xor)
                nc.vector.tensor_scalar(yq[:], yq[:], 0x5f3759e0, None, ALU.add)
                half = smp.tile([128, 4], F32, name=f"hf{tag}", tag="hf4")
                nc.vector.tensor_scalar_mul(half[:], var4[:], 0.5)
                yf = yq[:].bitcast(F32)
                for it in range(1):
                    t2 = smp.tile([128, 4], F32, name=f"t2{tag}{it}", tag="t24")
                    nc.vector.tensor_tensor(out=t2[:], in0=yf, in1=yf, op=ALU.mult)
                    nc.vector.tensor_tensor(out=t2[:], in0=t2[:], in1=half[:], op=ALU.mult)
                    nc.vector.tensor_scalar(t2[:], t2[:], -1.0, 1.5, ALU.mult, ALU.add)
                    nc.vector.tensor_tensor(out=yq[:].bitcast(F32), in0=yf, in1=t2[:], op=ALU.mult)
                if tail:
                    nmrst = smp.tile([128, 4], F32, name=f"nr{tag}", tag="nr4")
                    nc.vector.tensor_tensor(out=nmrst[:], in0=negmu[:], in1=yf, op=ALU.mult)
                for sub in range(4):
                    ti = qb * 4 + sub
                    if tail and sub % 2 == 0:
                        nc.scalar.activation(y[:, ti, :], x[:, ti, :], AF.Identity,
                                             bias=nmrst[:, sub:sub + 1],
                                             scale=yf[:, sub:sub + 1])
                    else:
                        eng = nc.gpsimd if sub % 2 else nc.vector
                        eng.tensor_scalar(y[:, ti, :], x[:, ti, :],
                                          negmu[:, sub:sub + 1], yf[:, sub:sub + 1],
                                          ALU.add, ALU.mult)
                    if store and not use_g:
                        nc.sync.dma_start(
                            dO.rearrange("(t p) e -> p t e", p=128)[:, ti:ti + 1, :],
                            y[:, ti:ti + 1, :])
                if use_g:
                    gB = cps.tile([128, 512], F32, name=f"gB{tag}", tag="pa", bufs=2)
                    nc.tensor.matmul(gB[:, 0:D], onesr1[:], vrows[gname][:], start=True, stop=True)
                    bB = cps.tile([128, 512], F32, name=f"bB{tag}", tag="pa", bufs=2)
                    nc.tensor.matmul(bB[:, 0:D], onesr1[:], vrows[bname][:], start=True, stop=True)
                    for sub in range(4):
                        ti = qb * 4 + sub
                        nc.vector.tensor_tensor(out=y[:, ti, :], in0=y[:, ti, :], in1=gB[:, 0:D], op=ALU.mult)
                        nc.vector.tensor_tensor(out=y[:, ti, :], in0=y[:, ti, :], in1=bB[:, 0:D], op=ALU.add)

            def ln_fin_sub(x, y, qb, sub, tag, sum4, sumsq4, act_half, store):
                """Per-sub LN finish: [128,1] micro chain so each sub-tile's
                pipeline completes independently (tail latency)."""
                ve = nc.vector
                ti = qb * 4 + sub
                sl = slice(sub, sub + 1)
                negmu = smp.tile([128, 1], F32, name=f"nm{tag}{sub}", tag="nm1")
                ve.tensor_scalar_mul(negmu[:], sum4[:, sl], -1.0 / D)
                var1 = smp.tile([128, 1], F32, name=f"va{tag}{sub}", tag="va1")
                ve.tensor_tensor(out=var1[:], in0=negmu[:], in1=negmu[:], op=ALU.mult)
                sq = smp.tile([128, 1], F32, name=f"sq{tag}{sub}", tag="sq1")
                ve.tensor_scalar_mul(sq[:], sumsq4[:, sl], 1.0 / D)
                ve.tensor_tensor(out=var1[:], in0=sq[:], in1=var1[:], op=ALU.subtract)
                ve.tensor_scalar_add(var1[:], var1[:], EPS)
                yq = smp.tile([128, 1], I32, name=f"yq{tag}{sub}", tag="yq1")
                ve.tensor_scalar(yq[:], var1[:].bitcast(I32), 1, None,
                                 ALU.logical_shift_right)
                ve.tensor_scalar(yq[:], yq[:], -1, None, ALU.bitwise_xor)
                ve.tensor_scalar(yq[:], yq[:], 0x5f3759e0, None, ALU.add)
                half = smp.tile([128, 1], F32, name=f"hf{tag}{sub}", tag="hf1")
                ve.tensor_scalar_mul(half[:], var1[:], 0.5)
                yf = yq[:].bitcast(F32)
                for it in range(1):
                    t2 = smp.tile([128, 1], F32, name=f"t2{tag}{sub}{it}", tag="t21")
                    ve.tensor_tensor(out=t2[:], in0=yf, in1=yf, op=ALU.mult)
                    ve.tensor_tensor(out=t2[:], in0=t2[:], in1=half[:], op=ALU.mult)
                    ve.tensor_scalar(t2[:], t2[:], -1.0, 1.5, ALU.mult, ALU.add)
                    ve.tensor_tensor(out=yq[:].bitcast(F32), in0=yf, in1=t2[:], op=ALU.mult)
                if act_half:
                    nmrst = smp.tile([128, 1], F32, name=f"nr{tag}{sub}", tag="nr1")
                    nc.vector.tensor_tensor(out=nmrst[:], in0=negmu[:], in1=yf, op=ALU.mult)
                    nc.scalar.activation(y[:, ti, :], x[:, ti, :], AF.Identity,
                                         bias=nmrst[:], scale=yf)
                else:
                    nc.vector.tensor_scalar(y[:, ti, :], x[:, ti, :],
                                            negmu[:], yf, ALU.add, ALU.mult)
                if store:
                    nc.sync.dma_start(
                        dO.rearrange("(t p) e -> p t e", p=128)[:, ti:ti + 1, :],
                        y[:, ti:ti + 1, :])

            # ---------------- projections + attention + MLP ----------------
            with (
                tc.tile_pool(name="psA", bufs=1, space="PSUM") as psA,
                tc.tile_pool(name="scps", bufs=2, space="PSUM") as scps,
                tc.tile_pool(name="accps", bufs=2, space="PSUM") as accps,
            ):
                cps = psA  # C-phase tiles share the A-phase pool space

                # --- phase A PE work: only the first-needed projections
                # inline; the rest drains as thunks under the exp stream ---
                def qproj(qb, et, act_epi):
                    qsl = slice(qb * 512, (qb + 1) * 512)
                    ps = psA.tile([128, 512], F32, name=f"qp{et}{qb}", tag="pa", bufs=2)
                    for dt in range(ET):
                        nc.tensor.matmul(
                            ps[:], w_r["WqT"][:, dt, et * 128:(et + 1) * 128],
                            QTt[:, dt, qsl], start=(dt == 0), stop=(dt == ET - 1))
                    dst = qT_bf[:, et, qsl]
                    if use_bq:
                        nc.scalar.activation(dst, ps[:], AF.Identity,
                                             bias=vcols["bq"][:, et:et + 1])
                    elif act_epi:
                        nc.scalar.copy(dst, ps[:])
                    else:
                        nc.vector.tensor_copy(dst, ps[:])

                def kproj(et, kb):
                    ksl = slice(kb * 512, (kb + 1) * 512)
                    ps = psA.tile([128, 512], F32, name=f"kp{kb}{et}", tag="pa", bufs=2)
                    for dt in range(ET):
                        nc.tensor.matmul(
                            ps[:], w_r["WkT"][:, dt, et * 128:(et + 1) * 128],
                            KTt[:, dt, ksl], start=(dt == 0), stop=(dt == ET - 1))
                    dst = kT_bf[:, et, ksl]
                    if use_bk:
                        nc.vector.tensor_scalar_add(dst, ps[:], vcols["bk"][:, et:et + 1])
                    else:
                        nc.vector.tensor_copy(dst, ps[:])

                qproj(0, 0, True)
                qproj(0, 1, True)
                kproj(0, 0)
                kproj(0, 1)

                pending = []
                for kb in (2, 3):
                    pending.append((0.4, lambda kb=kb: kproj(0, kb)))
                pending.append((0.4, lambda: qproj(1, 0, False)))
                pending.append((0.4, lambda: qproj(1, 1, False)))
                for kb in range(KB):
                    pending.append((0.4, lambda kb=kb: kproj(1, kb)))
                for kt in range(KT):
                    def vproj(kt=kt):
                        ps = psA.tile([128, 512], F32, name=f"vp{kt}", tag="pa", bufs=2)
                        for dt in range(ET):
                            nc.tensor.matmul(
                                ps[:, 0:D], KTt[:, dt, kt * 128:(kt + 1) * 128],
                                w_r["WvT"][:, dt, :],
                                start=(dt == 0), stop=(dt == ET - 1))
                        nc.vector.tensor_copy(
                            v_sb[:, kt, :, 0:DH],
                            ps[:, 0:D].rearrange("p (h e) -> p h e", e=DH))
                    pending.append((0.5, vproj))
                for t in range(TT):
                    def qnat(t=t):
                        ps = psA.tile([128, 512], F32, name=f"qn{t}", tag="pa", bufs=2)
                        n_steps = ET + (1 if (use_bq or use_bv) else 0)
                        for dt in range(ET):
                            nc.tensor.matmul(
                                ps[:, 0:D], QTt[:, dt, t * 128:(t + 1) * 128],
                                w_r["WqT"][:, dt, :],
                                start=(dt == 0), stop=(dt == n_steps - 1))
                        if use_bq or use_bv:
                            nc.tensor.matmul(ps[:, 0:D], onesr1[:], vrows["bqv"][:],
                                             start=False, stop=True)
                        nc.vector.tensor_copy(q_nat[:, t, :], ps[:, 0:D])
                    pending.append((0.5, qnat))

                # --- per-block tail thunks ---
                def bc_groups(qb, hs, tail=False):
                    """A@V accumulation groups for heads `hs` of block qb.
                    Tail groups spread across the idle score-pool banks so
                    more accumulations run concurrently."""
                    th = []
                    hold = {}
                    for idx, (sub, h) in enumerate(
                            (s, h) for s in range(4) for h in hs):
                        hp, hh = divmod(h, 2)
                        def g(idx=idx, sub=sub, h=h, hp=hp, hh=hh, tail=tail):
                            if tail:
                                if idx % 2 == 0:
                                    hold[idx // 2] = scps.tile(
                                        [128, 1024], F32, name=f"as{qb}{idx}", tag="sc")
                                base = hold[idx // 2]
                                acc = base[:, 0:DH + 1] if idx % 2 == 0 \
                                    else base[:, 512:512 + DH + 1]
                            else:
                                acc = accps.tile([128, DH + 1], F32,
                                                 name=f"ac{qb}{sub}{h}", tag="acc")
                            for kt in range(KT):
                                nc.tensor.matmul(
                                    acc,
                                    uts[:, hp * KT + kt,
                                        hh * 512 + sub * 128: hh * 512 + (sub + 1) * 128],
                                    v_sb[:, kt, h, :],
                                    start=(kt == 0), stop=(kt == KT - 1))
                            rec = smp.tile([128, 1], F32, name=f"rc{qb}{sub}{h}", tag="rec")
                            nc.vector.reciprocal_approx_fast(out=rec[:], in_=acc[:, DH:DH + 1])
                            if tail and (sub + h) % 2 == 0:
                                nc.scalar.activation(tmp_nrm[qb % 2][sub][:, h, :],
                                                     acc[:, 0:DH], AF.Copy, scale=rec[:])
                            else:
                                nc.vector.tensor_scalar_mul(
                                    tmp_nrm[qb % 2][sub][:, h, :], acc[:, 0:DH], rec[:])
                        th.append((0.8, g))
                    return th

                tmp_nrm = [[None] * 4 for _ in range(2)]

                def bc_rest(qb, l0s, l0q, l1s, l1q, tail=False):
                    th = []
                    cseq = [0]

                    def c_tile(name):
                        cseq[0] += 1
                        if tail and cseq[0] % 2 == 0:
                            return scps.tile([128, 1024], F32, name=name, tag="sc")
                        return cps.tile([128, 512], F32, name=name, tag="pa", bufs=2)
                    for sub in range(4):
                        def res(sub=sub, l0s=l0s, l0q=l0q, tail=tail):
                            ti = qb * 4 + sub
                            nc.gpsimd.tensor_tensor(
                                out=O0n[:, ti, :],
                                in0=tmp_nrm[qb % 2][sub][:].rearrange("p h e -> p (h e)"),
                                in1=q_nat[:, ti, :], op=ALU.add)
                            ln_stats(O0n, qb, sub, l0s, l0q, tail)
                        th.append((0.3, res))
                    th.append((0.3, lambda l0s=l0s, l0q=l0q: ln_finish(
                        O0n, O1n, qb, "g0", "b0", use_g0, f"l0q{qb}", l0s, l0q, cps, tail)))
                    # O1n -> O1T transposes (feeds FFN contraction)
                    for i in range(4):
                        ti = qb * 4 + i
                        def t1(ti=ti, tail=tail):
                            tp = c_tile(f"tpA{ti}")
                            for et in range(ET):
                                nc.tensor.transpose(tp[:, et * 128:(et + 1) * 128],
                                                    O1n[:, ti, et * 128:(et + 1) * 128], idn[:])
                            src = tp[:, 0:D].rearrange("p (et e) -> p et e", e=128)
                            dst = O1T[:, :, ti * 128:(ti + 1) * 128]
                            if tail and ti % 2 == 0:
                                nc.scalar.copy(dst, src)
                            else:
                                nc.vector.tensor_copy(dst, src)
                        th.append((0.5, t1))
                    # FFN matmuls in 256-wide halves + relu epilogue
                    for et in range(ET):
                        for hf in range(2):
                            tok0 = qb * 512 + hf * 256
                            def t2(et=et, tok0=tok0, tail=tail):
                                ff = c_tile(f"ff{et}{tok0}")
                                for dt in range(ET):
                                    nc.tensor.matmul(
                                        ff[:, 0:D], w_r["WoT"][:, dt, et * 128:(et + 1) * 128],
                                        O1T[:, dt, tok0:tok0 + 256],
                                        start=(dt == 0), stop=(dt == ET - 1))
                                dst = rT[:, et, tok0:tok0 + 256]
                                if tail and (et + tok0 // 256) % 2 == 0:
                                    nc.scalar.activation(
                                        dst, ff[:, 0:D], AF.Relu,
                                        bias=vcols["bo"][:, et:et + 1] if use_bo else 0.0)
                                else:
                                    nc.vector.tensor_scalar(
                                        dst, ff[:, 0:D],
                                        vcols["bo"][:, et:et + 1] if use_bo else 0.0,
                                        0.0, ALU.add, ALU.max)
                            th.append((0.5, t2))
                    # relu back to natural + second residual + LN1 stats
                    for i in range(4):
                        ti = qb * 4 + i
                        def t3(ti=ti, i=i, l1s=l1s, l1q=l1q, tail=tail):
                            tp = c_tile(f"tpB{ti}")
                            for et in range(ET):
                                nc.tensor.transpose(tp[:, et * 128:(et + 1) * 128],
                                                    rT[:, et, ti * 128:(ti + 1) * 128], idn[:])
                            nc.vector.tensor_tensor(out=O2n[:, ti, :], in0=O1n[:, ti, :],
                                                    in1=tp[:, 0:D], op=ALU.add)
                            ln_stats(O2n, qb, i, l1s, l1q, tail)
                        th.append((0.5, t3))

                    def fin(l1s=l1s, l1q=l1q):
                        per_sub = tail and not use_g1
                        ln_finish(O2n, O3n, qb, "g1", "b1", use_g1,
                                  f"l1q{qb}", l1s, l1q, cps, tail, store=per_sub)
                        if not per_sub:
                            nc.sync.dma_start(
                                dO.rearrange("(t p) e -> p t e", p=128)[:, qb * 4:(qb + 1) * 4, :],
                                O3n[:, qb * 4:(qb + 1) * 4, :])
                    th.append((0.5, fin))
                    return th

                def tail_chain(qb, l0s, l0q, l1s, l1q):
                    """Per-sub tail for the final block. Each of the 8
                    remaining A@V groups gets its OWN psum bank (accps +
                    psA + scps are all idle post-stream) so accumulations
                    run fully parallel; each sub-tile then flows through
                    residual->LN0->transpose->FFN->LN1->store independently."""
                    th = []
                    hold = {}

                    def acc_for(sub, j):
                        n = sub * 2 + j
                        if n < 2:
                            return accps.tile([128, DH + 1], F32,
                                              name=f"ta{qb}{n}", tag="acc")
                        if n < 4:
                            t = cps.tile([128, 512], F32, name=f"ta{qb}{n}",
                                         tag="pa", bufs=2)
                            return t[:, 0:DH + 1]
                        pi = (n - 4) // 2
                        if (n - 4) % 2 == 0:
                            hold[pi] = scps.tile([128, 1024], F32,
                                                 name=f"ta{qb}{n}", tag="sc")
                            return hold[pi][:, 0:DH + 1]
                        return hold[pi][:, 512:512 + DH + 1]

                    def mk_g(sub, h, j):
                        hp, hh = divmod(h, 2)
                        def g():
                            acc = acc_for(sub, j)
                            for kt in range(KT):
                                nc.tensor.matmul(
                                    acc,
                                    uts[:, hp * KT + kt,
                                        hh * 512 + sub * 128: hh * 512 + (sub + 1) * 128],
                                    v_sb[:, kt, h, :],
                                    start=(kt == 0), stop=(kt == KT - 1))
                            rec = smp.tile([128, 1], F32, name=f"rc{qb}{sub}{h}", tag="rec")
                            nc.vector.reciprocal_approx_fast(out=rec[:], in_=acc[:, DH:DH + 1])
                            if (sub + h) % 2 == 0:
                                nc.scalar.activation(tmp_nrm[qb % 2][sub][:, h, :],
                                                     acc[:, 0:DH], AF.Copy, scale=rec[:])
                            else:
                                nc.vector.tensor_scalar_mul(
                                    tmp_nrm[qb % 2][sub][:, h, :], acc[:, 0:DH], rec[:])
                        return g

                    def mk_rf0(sub):
                        def rf0():
                            ti = qb * 4 + sub
                            nc.gpsimd.tensor_tensor(
                                out=O0n[:, ti, :],
                                in0=tmp_nrm[qb % 2][sub][:].rearrange("p h e -> p (h e)"),
                                in1=q_nat[:, ti, :], op=ALU.add)
                            ln_stats(O0n, qb, sub, l0s, l0q, tail=True)
                            ln_fin_sub(O0n, O1n, qb, sub, f"f0{qb}", l0s, l0q,
                                       act_half=(sub % 2 == 0), store=False)
                        return rf0

                    def mk_t1(sub):
                        def t1():
                            ti = qb * 4 + sub
                            tp = cps.tile([128, 512], F32, name=f"tlA{ti}", tag="pa", bufs=2)
                            for et in range(ET):
                                nc.tensor.transpose(tp[:, et * 128:(et + 1) * 128],
                                                    O1n[:, ti, et * 128:(et + 1) * 128], idn[:])
                            src2 = tp[:, 0:D].rearrange("p (et e) -> p et e", e=128)
                            dst = O1T[:, :, ti * 128:(ti + 1) * 128]
                            if sub % 2 == 0:
                                nc.scalar.copy(dst, src2)
                            else:
                                nc.vector.tensor_copy(dst, src2)
                        return t1

                    def mk_ffn(et, hf):
                        def t2():
                            tok0 = qb * 512 + hf * 256
                            ff = scps.tile([128, 1024], F32, name=f"tf{et}{hf}", tag="sc")
                            for dt in range(ET):
                                nc.tensor.matmul(
                                    ff[:, 0:D], w_r["WoT"][:, dt, et * 128:(et + 1) * 128],
                                    O1T[:, dt, tok0:tok0 + 256],
                                    start=(dt == 0), stop=(dt == ET - 1))
                            dst = rT[:, et, tok0:tok0 + 256]
                            if et % 2 == 0:
                                nc.scalar.activation(
                                    dst, ff[:, 0:D], AF.Relu,
                                    bias=vcols["bo"][:, et:et + 1] if use_bo else 0.0)
                            else:
                                nc.vector.tensor_scalar(
                                    dst, ff[:, 0:D],
                                    vcols["bo"][:, et:et + 1] if use_bo else 0.0,
                                    0.0, ALU.add, ALU.max)
                        return t2

                    def mk_t3fin(sub):
                        def t3():
                            ti = qb * 4 + sub
                            tp = cps.tile([128, 512], F32, name=f"tlB{ti}", tag="pa", bufs=2)
                            for et in range(ET):
                                nc.tensor.transpose(tp[:, et * 128:(et + 1) * 128],
                                                    rT[:, et, ti * 128:(ti + 1) * 128], idn[:])
                            nc.vector.tensor_tensor(out=O2n[:, ti, :], in0=O1n[:, ti, :],
                                                    in1=tp[:, 0:D], op=ALU.add)
                            ln_stats(O2n, qb, sub, l1s, l1q, tail=True)
                            ln_fin_sub(O2n, O3n, qb, sub, f"f1{qb}", l1s, l1q,
                                       act_half=(sub % 2 == 0), store=True)
                        return t3

                    for sub in range(4):
                        th.append((0.7, mk_g(sub, 2, 0)))
                        th.append((0.7, mk_g(sub, 3, 1)))
                        th.append((0.3, mk_rf0(sub)))
                    for sub in range(4):
                        th.append((0.4, mk_t1(sub)))
                        if sub % 2 == 1:
                            hf = sub // 2
                            th.append((0.4, mk_ffn(0, hf)))
                            th.append((0.4, mk_ffn(1, hf)))
                            th.append((0.5, mk_t3fin(sub - 1)))
                            th.append((0.5, mk_t3fin(sub)))
                    return th

                # --- main loop: exp stream with thunk draining ---
                for qb in range(QB):
                    qsl = slice(qb * 512, (qb + 1) * 512)
                    for sub in range(4):
                        tmp_nrm[qb % 2][sub] = smp.tile(
                            [128, 4, DH], F32, name=f"tmp{qb}{sub}", tag=f"tmp{sub}", bufs=2)
                    l0s = smp.tile([128, 4], F32, name=f"l0s{qb}", tag="l0s", bufs=2)
                    l0q = smp.tile([128, 4], F32, name=f"l0q{qb}", tag="l0q", bufs=2)
                    l1s = smp.tile([128, 4], F32, name=f"l1s{qb}", tag="l1s", bufs=2)
                    l1q = smp.tile([128, 4], F32, name=f"l1q{qb}", tag="l1q", bufs=2)

                    sc_tiles = {}

                    def emit_sc(i, qsl=qsl, sc_tiles=sc_tiles):
                        hp, kt = divmod(i, KT)
                        sct = scps.tile([128, 1024], F32, name=f"sc{qb}_{i}", tag="sc")
                        for hh in range(2):
                            off = hh * 64
                            nc.tensor.matmul(
                                sct[:, hh * 512:(hh + 1) * 512],
                                kT_bf[off:off + 64, hp, kt * 128:(kt + 1) * 128],
                                qT_bf[off:off + 64, hp, qsl],
                                start=True, stop=True)
                        sc_tiles[i] = sct

                    emit_sc(0)
                    for i in range(32):
                        nc.scalar.activation(uts[:, i, :], sc_tiles.pop(i)[:],
                                             AF.Exp, scale=SCALE)
                        if i + 1 < 32:
                            emit_sc(i + 1)
                        if i >= 2 or qb > 0:
                            used = 0.0
                            drained = 0
                            while pending and (drained == 0 or
                                               used + pending[0][0] <= 1.01):
                                c, f = pending.pop(0)
                                f()
                                used += c
                                drained += 1
                        if i == 16:
                            pending.extend(bc_groups(qb, (0, 1), tail=False))
                    last = qb == QB - 1
                    if last and not (use_g0 or use_g1):
                        pending.extend(tail_chain(qb, l0s, l0q, l1s, l1q))
                    else:
                        pending.extend(bc_groups(qb, (2, 3), tail=False))
                        pending.extend(bc_rest(qb, l0s, l0q, l1s, l1q, tail=False))

                for c, f in pending:
                    f()

    nc.compile()
    return nc


def kernel(Q, K, Wq, bq, Wk, bk, Wv, bv, Wo, bo, g0, b0, g1, b1):
    Q, K = np.asarray(Q), np.asarray(K)
    ws = {n: np.ascontiguousarray(np.asarray(v).T, dtype=np.float32)
          for n, v in (("WqT", Wq), ("WkT", Wk), ("WvT", Wv), ("WoT", Wo))}
    vs = {n: np.ascontiguousarray(np.asarray(v), dtype=np.float32)
          for n, v in (("bq", bq), ("bk", bk), ("bv", bv), ("bo", bo),
                       ("g0", g0), ("b0", b0), ("g1", g1), ("b1", b1))}
    flags = (bool(np.any(vs["bq"])), bool(np.any(vs["bk"])),
             bool(np.any(vs["bv"])), bool(np.any(vs["bo"])),
             bool(np.any(vs["g0"] != 1.0) or np.any(vs["b0"])),
             bool(np.any(vs["g1"] != 1.0) or np.any(vs["b1"])))
    if flags not in _CACHE:
        _CACHE[flags] = _build(flags)
    nc = _CACHE[flags]

    idn = np.eye(128, dtype=np.float32)
    kts = [np.ascontiguousarray(K[b].T, dtype=np.float32) for b in range(B)]
    in_maps = []
    for b in range(B):
        for half in range(2):
            m = {"QT": np.ascontiguousarray(Q[b, half * S:(half + 1) * S].T, dtype=np.float32),
                 "KT": kts[b], "IDN": idn}
            m.update(ws)
            m.update(vs)
            in_maps.append(m)

    res = run_bass_kernel_spmd(nc, in_maps, list(range(8)))
    out = np.empty((B, NQ, D), dtype=np.float32)
    for i in range(8):
        b, half = divmod(i, 2)
        out[b, half * S:(half + 1) * S] = res.results[i]["Out"]
    return out
